# revision 31
# baseline (speedup 1.0000x reference)
"""DGMC (deep graph matching consensus) forward pass on 8 Trainium2 cores.

Sharding: core c handles graph b = c//2, source-row half h = c%2 (512 of 1024
source rows). All-pairs work, softmaxes and outputs are s-sharded; the only
cross-core communication is an AllReduce of the [16,1024] r_t partial within
each 2-core pair, once per refinement step.

Key restructurings vs the reference:
 - segment_sum GraphConv aggregations -> dense adjacency matmuls (host builds
   the [src,dst] count matrices; exact in fp32/fp32r since counts are ints).
 - The [B,Ns,Nt,R] relu-MLP:
     upd[s,t] = sum_r w_r relu((o_s@Wm1)[s,r] - (o_t@Wm1)[t,r] + bm1[r]) + bm2
              = sum_r sgn(w_r) relu(Q[s,r] + T[t,r]) + bm2,
   with Q = o_s@Wm1pos + bm1*|w|, T = -(o_t@Wm1pos), Wm1pos = Wm1*diag(|w|).
   T is materialized replicated 8x across partitions (p = s8*16+r) as fp16;
   one fused DVE tensor_scalar (add + max0) per 8-source-row group produces
   relu(Q + T) and PE matmuls with +-1 selection matrices do the r-sum,
   accumulating [128,1024] update tiles in PSUM.  bm2 adds the same constant
   to every logit of a row, so it is dropped (softmax-invariant).
 - softmax: DVE negated row-max + ACT exp with fused bias and fused row-sum;
   the 1/rowsum normalization is folded into r_s for the S^T r_s matmul and
   only applied explicitly for the two output softmaxes.
"""

import numpy as np

import concourse.mybir as mybir
import concourse.tile as tile
from concourse import bacc
from concourse.bass_utils import run_bass_kernel_spmd

F32 = mybir.dt.float32
F32R = mybir.dt.float32r
F16 = mybir.dt.float16
ALU = mybir.AluOpType
ACTF = mybir.ActivationFunctionType
AXX = mybir.AxisListType.X

B, N, F, R, DEG, STEPS = 4, 1024, 128, 16, 16, 2
NH = N // 2          # source rows per core
NT = NH // 128       # source-row tiles per core (4)
KT = N // 128        # contraction tiles over a full graph (8)
NCORES = 8


def _f32r(x):
    """Round to FP22 (what the PE reads for float32r) so the BIR verifier's
    pre-rounded requirement is honest; round-to-nearest on the dropped bits."""
    u = np.ascontiguousarray(x, np.float32).view(np.uint32)
    u = (u + np.uint32(0x200)) & np.uint32(0xFFFFFC00)
    return u.view(np.float32)


def _build_program():
    nc = bacc.Bacc("TRN2", target_bir_lowering=False, debug=False,
                   num_devices=NCORES)

    def din(name, shape, dt):
        return nc.dram_tensor(name, shape, dt, kind="ExternalInput").ap()

    def dout(name, shape, dt):
        return nc.dram_tensor(name, shape, dt, kind="ExternalOutput").ap()

    xsT = din("xsT", [F, N], F32)          # x_s[graph]^T
    xsTh = din("xsTh", [F, NH], F32)       # x_s^T, this core's dst half
    xtT = din("xtT", [F, N], F32)
    asT = din("asT", [N, NH], F32R)        # A_s^T[src, dst-half], counts
    atT = din("atT", [N, N], F32R)         # A_t^T[src, dst], counts
    rsN = din("rsN", [STEPS, NH, R], F32R)  # r_s rows for this core's half
    rsT = din("rsT", [STEPS, R, N], F32R)  # r_s^T full graph
    rsTh = din("rsTh", [STEPS, R, NH], F32R)  # r_s^T, this core's dst half
    w1s = din("w1s", [F, F], F32)
    w1n = din("w1n", [F, F], F32)
    b1 = din("b1", [F, 1], F32)
    w2s = din("w2s", [R, R], F32R)
    w2n = din("w2n", [R, R], F32R)
    b2 = din("b2", [R, 1], F32)
    wm1p = din("wm1p", [R, R], F32R)       # Wm1 * diag(|Wm2|)
    negrep = din("negrep", [R, 128], F32R)  # -Wm1pos replicated 8x over cols
    cpos = din("cpos", [R, 1], F32)        # bm1 * |Wm2|
    sg = din("sg", [16, 128, 128], F16)    # per-group +-1 selection matrices
    o_s0 = dout("s0h", [NH, N], F32)
    o_sl = dout("slh", [NH, N], F32)

    with tile.TileContext(nc) as tc:
        with tc.tile_pool(name="cst", bufs=1) as cst, \
             tc.tile_pool(name="big", bufs=1) as bigp, \
             tc.tile_pool(name="shat", bufs=8) as shatp, \
             tc.tile_pool(name="ep", bufs=4) as ep, \
             tc.tile_pool(name="hp", bufs=6) as hp, \
             tc.tile_pool(name="trp", bufs=2) as trp, \
             tc.tile_pool(name="qp", bufs=8) as qp, \
             tc.tile_pool(name="sm", bufs=4) as smp, \
             tc.tile_pool(name="md", bufs=2) as mdp, \
             tc.tile_pool(name="sm1", bufs=1) as sm1, \
             tc.tile_pool(name="outp", bufs=3) as outp, \
             tc.tile_pool(name="ps4", bufs=4, space="PSUM") as ps4, \
             tc.tile_pool(name="ps2", bufs=2, space="PSUM") as ps2, \
             tc.tile_pool(name="dram", bufs=1, space="DRAM") as drp:

            # ---------------- load constants / inputs ----------------
            t_xsT = cst.tile([F, N], F32, tag="xsT")
            t_xsTh = cst.tile([F, NH], F32, tag="xsTh")
            t_xtT = cst.tile([F, N], F32, tag="xtT")
            t_as = cst.tile([128, KT, NH], F32R, tag="asT")
            t_at = cst.tile([128, KT, N], F32R, tag="atT")
            t_rsN = cst.tile([128, STEPS * NT, R], F32R, tag="rsN")
            t_rsT = cst.tile([R, STEPS, N], F32R, tag="rsT")
            t_rsTh = cst.tile([R, STEPS, NH], F32R, tag="rsTh")
            t_w1s = cst.tile([F, F], F32, tag="w1s")
            t_w1n = cst.tile([F, F], F32, tag="w1n")
            t_b1 = cst.tile([F, 1], F32, tag="b1")
            t_w2s = cst.tile([R, R], F32R, tag="w2s")
            t_w2n = cst.tile([R, R], F32R, tag="w2n")
            t_b2 = cst.tile([R, 1], F32, tag="b2")
            t_wm1p = cst.tile([R, R], F32R, tag="wm1p")
            t_negrep = cst.tile([R, 128], F32R, tag="negrep")
            t_cpos = cst.tile([R, 1], F32, tag="cpos")
            t_sg = cst.tile([128, 16, 128], F16, tag="sg")

            nc.sync.dma_start(t_xsT[:], xsT[:])
            nc.sync.dma_start(t_xsTh[:], xsTh[:])
            nc.sync.dma_start(t_xtT[:], xtT[:])
            # split A loads per k-tile so psi1 agg matmuls can chase the DMA;
            # use two different engines' HW queues to run them in parallel
            for k in range(KT):
                nc.sync.dma_start(t_as[:, k, :],
                                  asT[k * 128:(k + 1) * 128, :])
                nc.gpsimd.dma_start(t_at[:, k, :],
                                    atT[k * 128:(k + 1) * 128, :])
            nc.sync.dma_start(
                t_rsN[:], rsN.rearrange("s (t p) r -> p (s t) r", p=128))
            nc.sync.dma_start(t_rsT[:], rsT.rearrange("s r n -> r s n"))
            nc.sync.dma_start(t_rsTh[:], rsTh.rearrange("s r n -> r s n"))
            nc.sync.dma_start(t_w1s[:], w1s[:])
            nc.sync.dma_start(t_w1n[:], w1n[:])
            nc.sync.dma_start(t_b1[:], b1[:])
            nc.sync.dma_start(t_w2s[:], w2s[:])
            nc.sync.dma_start(t_w2n[:], w2n[:])
            nc.sync.dma_start(t_b2[:], b2[:])
            nc.sync.dma_start(t_wm1p[:], wm1p[:])
            nc.sync.dma_start(t_negrep[:], negrep[:])
            nc.sync.dma_start(t_cpos[:], cpos[:])
            nc.sync.dma_start(t_sg[:], sg.rearrange("g p m -> p g m"))

            # ---------------- psi1 (split-fp32r, exact) ----------------
            # xW_nbr for both graphs per 128-src tile, split into two fp32r
            # tensors (hi + residual) so the big A-streaming matmuls run at
            # 1 cyc/col with exact products (A counts are FP22-exact ints).
            t_xwn_s1 = bigp.tile([128, KT, F], F32R, tag="xwn_s1")
            t_xwn_s2 = bigp.tile([128, KT, F], F32R, tag="xwn_s2")
            t_xwn_t1 = bigp.tile([128, KT, F], F32R, tag="xwn_t1")
            t_xwn_t2 = bigp.tile([128, KT, F], F32R, tag="xwn_t2")
            for (xT, d1, d2) in ((t_xsT, t_xwn_s1, t_xwn_s2),
                                 (t_xtT, t_xwn_t1, t_xwn_t2)):
                for k in range(KT):
                    p = ps4.tile([128, 512], F32, tag="pbig")
                    nc.tensor.matmul(p[:, :F], xT[:, k * 128:(k + 1) * 128],
                                     t_w1n[:], start=True, stop=True)
                    nc.scalar.copy(d1[:, k, :], p[:, :F])
                    nc.vector.tensor_tensor(d2[:, k, :], p[:, :F],
                                            d1[:, k, :], ALU.subtract)

            # h^T = relu(W1s^T xT + xWn^T A^T + b1), transposed layout [f', n]
            t_hs = bigp.tile([F, NH], F32, tag="hs")
            t_ht = bigp.tile([F, N], F32, tag="ht")
            # self-term rhs: dst rows of each chunk (s: this core's half)
            for (selfx, xw1, xw2, amat, hout, nchunks) in (
                    (t_xsTh, t_xwn_s1, t_xwn_s2, t_as, t_hs, 1),
                    (t_xtT, t_xwn_t1, t_xwn_t2, t_at, t_ht, 2)):
                for c in range(nchunks):
                    p = ps4.tile([128, 512], F32, tag="pbig")
                    nc.tensor.matmul(p[:], t_w1s[:],
                                     selfx[:, 512 * c:512 * (c + 1)],
                                     start=True, stop=False)
                    for k in range(KT):
                        nc.tensor.matmul(
                            p[:], xw1[:, k, :],
                            amat[:, k, 512 * c:512 * (c + 1)],
                            start=False, stop=False, skip_group_check=True)
                        nc.tensor.matmul(
                            p[:], xw2[:, k, :],
                            amat[:, k, 512 * c:512 * (c + 1)],
                            start=False, stop=(k == KT - 1),
                            skip_group_check=True)
                    nc.scalar.activation(hout[:, 512 * c:512 * (c + 1)],
                                         p[:], ACTF.Relu, bias=t_b1[:])

            # ---------------- S_hat (fp32, exact) ----------------
            shat = []
            for i in range(NT):
                st_t = shatp.tile([128, N], F32, tag="shat")
                for c in range(2):
                    p = ps4.tile([128, 512], F32, tag="pbig")
                    nc.tensor.matmul(p[:], t_hs[:, i * 128:(i + 1) * 128],
                                     t_ht[:, 512 * c:512 * (c + 1)],
                                     start=True, stop=True)
                    nc.scalar.copy(st_t[:, 512 * c:512 * (c + 1)], p[:])
                shat.append(st_t)

            # ---------------- psi2 on r_s (both steps, input-only) --------
            q_tiles = [[None] * NT for _ in range(STEPS)]
            for s in range(STEPS):
                # rW2n_s[src, r'] per k-tile
                t_rws = mdp.tile([128, KT, R], F32R, tag="rws")
                for k in range(KT):
                    p = ps2.tile([128, 512], F32, tag="psm")
                    nc.tensor.matmul(p[:, :R],
                                     t_rsT[:, s, k * 128:(k + 1) * 128],
                                     t_w2n[:], start=True, stop=True)
                    nc.scalar.copy(t_rws[:, k, :], p[:, :R])
                # o_s^T[r', s-half] = relu(W2s^T rsT_half + rws^T A_s^T + b2)
                p = ps2.tile([128, 512], F32, tag="psm")
                nc.tensor.matmul(p[:R, :], t_w2s[:], t_rsTh[:, s, :],
                                 start=True, stop=False)
                for k in range(KT):
                    nc.tensor.matmul(p[:R, :], t_rws[:, k, :],
                                     t_as[:, k, :], start=False,
                                     stop=(k == KT - 1))
                t_ost = mdp.tile([R, NH], F32R, tag="ost")
                nc.scalar.activation(t_ost[:], p[:R, :], ACTF.Relu,
                                     bias=t_b2[:])
                # Q^T[r', s-half] = Wm1pos^T o_sT + cpos
                p2 = ps2.tile([128, 512], F32, tag="psm")
                nc.tensor.matmul(p2[:R, :], t_wm1p[:], t_ost[:],
                                 start=True, stop=True)
                t_pst = mdp.tile([R, NH], F32, tag="pst")
                nc.scalar.activation(t_pst[:], p2[:R, :], ACTF.Identity,
                                     bias=t_cpos[:])
                # scramble via DRAM bounce: write PsT in Q order, read back
                # contiguously.  Qd[(i), p=(s8*16+r), g] = PsT[r, i*128+g*8+s8]
                for i in range(NT):
                    qd = drp.tile([128, 16], F32, tag=f"qd{s}_{i}",
                                  name=f"qd{s}_{i}")
                    nc.sync.dma_start(
                        qd[:].rearrange("(s8 r) g -> r g s8", s8=8, r=16),
                        t_pst[:, i * 128:(i + 1) * 128].rearrange(
                            "r (g s8) -> r g s8", g=16, s8=8))
                    qt = qp.tile([128, 16], F32, tag="q", name=f"q{s}_{i}")
                    nc.sync.dma_start(qt[:], qd[:])
                    q_tiles[s][i] = qt

            # ---------------- refinement steps ----------------
            cur_shat = shat
            rg = [[2 * g, 2 * g + 1] for g in range(NCORES // 2)]
            for s in range(STEPS):
                # softmax pieces + r_t partial
                p_rt = [ps2.tile([R, 512], F32, tag="prt", name=f"prt{s}_{c}")
                        for c in range(2)]
                e_tiles = []
                beta = []
                for i in range(NT):
                    nm = smp.tile([128, 1], F32, tag="nm")
                    nc.vector.tensor_reduce(nm[:], cur_shat[i][:], AXX,
                                            ALU.max, negate=True)
                    et = ep.tile([128, N], F32R, tag="E")
                    rs_sum = smp.tile([128, 1], F32, tag="rs_sum")
                    nc.scalar.activation(et[:], cur_shat[i][:], ACTF.Exp,
                                         bias=nm[:], accum_out=rs_sum[:])
                    bt = smp.tile([128, 1], F32, tag="beta")
                    nc.vector.reciprocal(bt[:], rs_sum[:])
                    e_tiles.append(et)
                    beta.append(bt)
                    rf = smp.tile([128, R], F32R, tag="rfold")
                    nc.vector.tensor_scalar(rf[:], t_rsN[:, s * NT + i, :],
                                            bt[:], None, ALU.mult)
                    for c in range(2):
                        nc.tensor.matmul(p_rt[c][:],
                                         rf[:],
                                         et[:, 512 * c:512 * (c + 1)],
                                         start=(i == 0), stop=(i == NT - 1))
                    if s == 0:
                        so = outp.tile([128, N], F32, tag="sout")
                        nc.vector.tensor_scalar(so[:], et[:].bitcast(F32),
                                                bt[:], None, ALU.mult)
                        nc.sync.dma_start(
                            o_s0[i * 128:(i + 1) * 128, :], so[:])

                # r_t partial -> pairwise AllGather -> local add -> r_t^T
                t_rtp = sm1.tile([R, N], F32, tag="rtp")
                for c in range(2):
                    nc.scalar.copy(t_rtp[:, 512 * c:512 * (c + 1)],
                                   p_rt[c][:])
                d_in = drp.tile([R, N], F32, tag="d_in")
                d_out = drp.tile([2 * R, N], F32, tag="d_out")
                nc.sync.dma_start(d_in[:], t_rtp[:])
                nc.gpsimd.collective_compute(
                    "AllGather", ALU.bypass, replica_groups=rg,
                    ins=[d_in.opt()], outs=[d_out.opt()])
                t_rtf = sm1.tile([R, 2, N], F32, tag="rtf")
                nc.sync.dma_start(
                    t_rtf[:], d_out.rearrange("(j p) n -> p j n", j=2))
                t_rt = sm1.tile([R, N], F32R, tag="rt")
                nc.vector.tensor_tensor(t_rt[:], t_rtf[:, 0, :],
                                        t_rtf[:, 1, :], ALU.add)

                # psi2 on r_t
                t_rwt = mdp.tile([128, KT, R], F32R, tag="rwt")
                for k in range(KT):
                    p = ps2.tile([128, 512], F32, tag="psm")
                    nc.tensor.matmul(p[:, :R],
                                     t_rt[:, k * 128:(k + 1) * 128],
                                     t_w2n[:], start=True, stop=True)
                    nc.scalar.copy(t_rwt[:, k, :], p[:, :R])
                t_ott = sm1.tile([R, N], F32R, tag="ott")
                for c in range(2):
                    p = ps2.tile([128, 512], F32, tag="psm")
                    nc.tensor.matmul(p[:R, :], t_w2s[:],
                                     t_rt[:, 512 * c:512 * (c + 1)],
                                     start=True, stop=False)
                    for k in range(KT):
                        nc.tensor.matmul(p[:R, :], t_rwt[:, k, :],
                                         t_at[:, k, 512 * c:512 * (c + 1)],
                                         start=False, stop=(k == KT - 1))
                    nc.scalar.activation(t_ott[:, 512 * c:512 * (c + 1)],
                                         p[:R, :], ACTF.Relu, bias=t_b2[:])
                # T_rep[p, t] = -(o_t @ Wm1pos)[t, r(p)], fp16, replicated 8x
                t_trep = trp.tile([128, N], F16, tag="trep")
                for c in range(2):
                    p = ps4.tile([128, 512], F32, tag="pbig")
                    nc.tensor.matmul(p[:], t_negrep[:],
                                     t_ott[:, 512 * c:512 * (c + 1)],
                                     start=True, stop=True)
                    nc.scalar.copy(t_trep[:, 512 * c:512 * (c + 1)], p[:])

                # pairwise relu + r-sum + S_hat update
                new_shat = []
                for i in range(NT):
                    pu = [ps4.tile([128, 512], F32, tag="pbig",
                                   name=f"pu{s}_{i}_{c}") for c in range(2)]
                    for g in range(16):
                        ht = hp.tile([128, N], F16, tag="H")
                        nc.vector.tensor_scalar(ht[:], t_trep[:],
                                                q_tiles[s][i][:, g:g + 1],
                                                0.0, ALU.add, ALU.max)
                        for c in range(2):
                            nc.tensor.matmul(pu[c][:], t_sg[:, g, :],
                                             ht[:, 512 * c:512 * (c + 1)],
                                             start=(g == 0), stop=(g == 15))
                    st_t = shatp.tile([128, N], F32, tag="shat")
                    for c in range(2):
                        nc.vector.tensor_tensor(
                            st_t[:, 512 * c:512 * (c + 1)], pu[c][:],
                            cur_shat[i][:, 512 * c:512 * (c + 1)], ALU.add)
                    new_shat.append(st_t)
                cur_shat = new_shat

            # ---------------- final softmax -> S_L ----------------
            for i in range(NT):
                nm = smp.tile([128, 1], F32, tag="nm")
                nc.vector.tensor_reduce(nm[:], cur_shat[i][:], AXX,
                                        ALU.max, negate=True)
                et = ep.tile([128, N], F32R, tag="E")
                rs_sum = smp.tile([128, 1], F32, tag="rs_sum")
                nc.scalar.activation(et[:], cur_shat[i][:], ACTF.Exp,
                                     bias=nm[:], accum_out=rs_sum[:])
                bt = smp.tile([128, 1], F32, tag="beta")
                nc.vector.reciprocal(bt[:], rs_sum[:])
                so = outp.tile([128, N], F32, tag="sout")
                nc.vector.tensor_scalar(so[:], et[:].bitcast(F32), bt[:],
                                        None, ALU.mult)
                nc.sync.dma_start(o_sl[i * 128:(i + 1) * 128, :], so[:])

    nc.compile()
    return nc


_PROGRAM = None


def _get_program():
    global _PROGRAM
    if _PROGRAM is None:
        _PROGRAM = _build_program()
    return _PROGRAM


def _host_prep(inputs):
    x_s = np.asarray(inputs["x_s"], np.float32)
    x_t = np.asarray(inputs["x_t"], np.float32)
    ei_s = np.asarray(inputs["edge_index_s"])
    ei_t = np.asarray(inputs["edge_index_t"])
    W1s = np.asarray(inputs["W1_self"], np.float32)
    W1n = np.asarray(inputs["W1_nbr"], np.float32)
    b1 = np.asarray(inputs["b1"], np.float32)
    W2s = np.asarray(inputs["W2_self"], np.float32)
    W2n = np.asarray(inputs["W2_nbr"], np.float32)
    b2 = np.asarray(inputs["b2"], np.float32)
    Wm1 = np.asarray(inputs["Wm1"], np.float32)
    bm1 = np.asarray(inputs["bm1"], np.float32)
    Wm2 = np.asarray(inputs["Wm2"], np.float32)
    bm2 = np.asarray(inputs["bm2"], np.float32)
    r_steps = np.asarray(inputs["r_steps"], np.float32)

    w = Wm2[:, 0]
    absw = np.abs(w)
    wm1p = Wm1 * absw[None, :]
    cpos = (bm1 * absw).reshape(R, 1)
    # negrep[r, s8*16+r'] = -wm1p[r, r']
    negrep = np.tile(-wm1p[:, None, :], (1, 8, 1)).reshape(R, 128)
    sgn = np.sign(w).astype(np.float16)
    sg = np.zeros((16, 128, 128), np.float16)
    for g in range(16):
        for s8 in range(8):
            for r in range(R):
                sg[g, s8 * 16 + r, g * 8 + s8] = sgn[r]

    def adj_T(ei, b):
        # A^T[src, dst] = #edges src->dst within graph b (local indices)
        lo, hi = b * N, (b + 1) * N
        m = (ei[1] >= lo) & (ei[1] < hi)
        src = ei[0][m] - lo
        dst = ei[1][m] - lo
        a = np.zeros((N, N), np.float32)
        np.add.at(a, (src, dst), 1.0)
        return a

    in_maps = []
    for c in range(NCORES):
        b, h = divmod(c, 2)
        As = adj_T(ei_s, b)
        At = adj_T(ei_t, b)
        rs_b = r_steps[:, b]  # [STEPS, N, R]
        xsT_b = np.ascontiguousarray(x_s[b * N:(b + 1) * N].T)
        rsT_b = _f32r(rs_b.transpose(0, 2, 1))
        in_maps.append({
            "xsT": xsT_b,
            "xsTh": np.ascontiguousarray(xsT_b[:, h * NH:(h + 1) * NH]),
            "xtT": np.ascontiguousarray(x_t[b * N:(b + 1) * N].T),
            "asT": _f32r(As[:, h * NH:(h + 1) * NH]),
            "atT": _f32r(At),
            "rsN": _f32r(rs_b[:, h * NH:(h + 1) * NH, :]),
            "rsT": rsT_b,
            "rsTh": np.ascontiguousarray(rsT_b[:, :, h * NH:(h + 1) * NH]),
            "w1s": W1s, "w1n": W1n, "b1": b1.reshape(F, 1),
            "w2s": _f32r(W2s), "w2n": _f32r(W2n), "b2": b2.reshape(R, 1),
            "wm1p": _f32r(wm1p), "negrep": _f32r(negrep),
            "cpos": cpos, "sg": sg,
        })
    return in_maps


_LAST_RESULT = None


def kernel(**inputs):
    global _LAST_RESULT
    nc = _get_program()
    in_maps = _host_prep(inputs)
    res = run_bass_kernel_spmd(nc, in_maps, list(range(NCORES)))
    _LAST_RESULT = res
    S0 = np.zeros((B, N, N), np.float32)
    SL = np.zeros((B, N, N), np.float32)
    for c in range(NCORES):
        b, h = divmod(c, 2)
        S0[b, h * NH:(h + 1) * NH] = res.results[c]["s0h"]
        SL[b, h * NH:(h + 1) * NH] = res.results[c]["slh"]
    return (S0, SL)
